# revision 14
# baseline (speedup 1.0000x reference)
"""Trainium2 Bass kernel for nn_CrossAttention (self-attention, B=2 N=4096 D=512 H=8 DH=64).

Sharding: 8 cores = 2 batches x 4 query-row slices (1024 rows each). Every core
projects the full 4096-token K/V for its batch (cheap, avoids collectives) and
computes attention + output projection for its 1024 query rows on-chip.

Single fused pass: for each 512-token x chunk, project K/V and immediately run
the attention j-step for all (query-tile, head-pair) against that chunk, so the
Activation engine (exp, the bottleneck at ~33.5M exps/core) starts early and
stays saturated. PV matmuls run transposed (out[q,65] = ex[tok,q].T @ v[tok,65])
in fp16 for full PE-column utilization; per-(q,head) sumexp rides along as the
65th v column.

PSUM layout (8 banks): 2 x [128,1024] score tiles (2 banks each) + one rotating
[128,4,512] 4-bank slot. A PSUM bank holds exactly ONE matmul accumulation group
(start=True zeroes the whole bank), so each of the 4 concurrent PV groups
(2 qc x 2 heads) accumulates in its own bank; the 8 (qc, head) groups per
(query-tile, head-pair) are processed as two sequential phases with a batched
DVE drain-add into an SBUF accumulator between them. The same 4-bank slot is
sub-aliased for x-transposes, K/V projection psums, and the epilogue.
"""

import os
import sys
from contextlib import ExitStack

import numpy as np

for _p in ("/opt/trn_rl_repo", "/root/.axon_site/_ro/trn_rl_repo"):
    if os.path.isdir(_p) and _p not in sys.path:
        sys.path.insert(0, _p)

import concourse.bass as bass
from concourse import bacc
import concourse.mybir as mybir
import concourse.tile as tile
from concourse.bass_utils import run_bass_kernel_spmd
from concourse.masks import make_identity

F32 = mybir.dt.float32
F32R = mybir.dt.float32r
F16 = mybir.dt.float16
EXP = mybir.ActivationFunctionType.Exp

# Problem dims (hardcoded per spec)
B, N, D = 2, 4096, 512
H, DH = 8, 64
SCALE = DH ** -0.5
NCORES = 8
CORES_PER_B = NCORES // B      # 4
NQ = N // CORES_PER_B          # 1024 query rows per core

NCH = N // 512                 # 8 x-chunks of 512 tokens
NJC = N // 128                 # 32 128-token j-chunks
NQT = NQ // 512                # 2 query tiles


def build_nc(debug=False):
    nc = bacc.Bacc(None, target_bir_lowering=False)
    x_d = nc.dram_tensor("x", [N, D], F32, kind="ExternalInput")
    xq_d = nc.dram_tensor("xq", [NQ, D], F32, kind="ExternalInput")
    wq_d = nc.dram_tensor("wq", [D, D], F32, kind="ExternalInput")
    wk_d = nc.dram_tensor("wk", [D, D], F32, kind="ExternalInput")
    wv_d = nc.dram_tensor("wv", [D, D], F32, kind="ExternalInput")
    wo_d = nc.dram_tensor("wo", [D, D], F32, kind="ExternalInput")
    bo_d = nc.dram_tensor("bo", [D], F32, kind="ExternalInput")
    out_d = nc.dram_tensor("out", [NQ, D], F32, kind="ExternalOutput")
    if debug:
        dbg_kT = nc.dram_tensor("dbg_kT", [128, N], F16, kind="ExternalOutput")
        dbg_qT = nc.dram_tensor("dbg_qT", [128, NQ], F16, kind="ExternalOutput")
        dbg_v = nc.dram_tensor("dbg_v", [128, NJC * H * 65], F16, kind="ExternalOutput")
        dbg_acc = nc.dram_tensor("dbg_acc", [128, NQT * 4 * 2 * 260], F32,
                                 kind="ExternalOutput")

    def dram_chunks(t):
        """View a [rows, 512] dram tensor as [128, rows//128, 512] for one DMA."""
        return t[:].rearrange("(c p) d -> p c d", p=128)

    with tile.TileContext(nc) as tc, ExitStack() as ctx:
        persist = ctx.enter_context(tc.tile_pool(name="persist", bufs=1))

        # Persistent SBUF state
        kT = [persist.tile([128, N], F16, tag=f"kT{i}", name=f"kT{i}") for i in range(4)]
        qT = [persist.tile([128, NQ], F16, tag=f"qT{i}", name=f"qT{i}") for i in range(4)]
        vsb = persist.tile([128, NJC, H, 65], F16, tag="vsb")   # [tok, j, head, 64+ones]
        wk_sb = persist.tile([128, 4, 512], F32R, tag="wk")
        wv_sb = persist.tile([128, 4, 512], F32R, tag="wv")
        wo_sb = persist.tile([128, 4, 512], F32R, tag="wo")
        bo_bc = persist.tile([128, 512], F32, tag="bo_bc")
        # acc[q, qt, hp, phase, (2qc x 2heads x 65)] running attention output
        acc = persist.tile([128, NQT, 4, 2, 260], F32, tag="acc")
        ident = persist.tile([128, 128], F32, tag="ident")
        ident_r = persist.tile([128, 128], F32R, tag="ident_r")

        make_identity(nc, ident)
        nc.vector.tensor_copy(ident_r, ident)
        # ones column of v_aug (col 64 of each head block)
        nc.vector.memset(vsb[:, :, :, 64:65], 1.0)
        # bias broadcast [512] -> [128, 512] via 0-stride DMA
        bo_t = bo_d.tensor if hasattr(bo_d, "tensor") else bo_d
        bo_ap = bass.AP(tensor=bo_t, offset=0, ap=[[0, 128], [1, 512]])
        nc.gpsimd.dma_start(out=bo_bc, in_=bo_ap)

        pools = [
            tc.tile_pool(name="xin", bufs=2),            # [128, 4, 512] f32r x staging
            tc.tile_pool(name="xts", bufs=8),            # [128, 512] f32r transposed x
            tc.tile_pool(name="exp", bufs=7),            # [128, 1024] f16 exp(scores)
            tc.tile_pool(name="oct", bufs=4),            # [128, 512] f32r attn-out.T
            tc.tile_pool(name="ocn", bufs=2),            # [128, 2,2,2,64] f32r normalized
            tc.tile_pool(name="rpp", bufs=2),            # [128, 2,2,2,1] f32 recip sumexp
            tc.tile_pool(name="outp", bufs=1),           # [128, 4, 512] f32 output staging
            tc.tile_pool(name="stp", bufs=2, space="PSUM"),   # [128,1024] scores, 2 banks ea
            tc.tile_pool(name="bigp", bufs=1, space="PSUM"),  # [128,4,512] 4-bank work slot
        ]
        xinp, xtsp, expp, octp, ocnp, rppp, outpp, stp, bigp = [
            ctx.enter_context(p) for p in pools]

        def big_tile(name):
            return bigp.tile([128, 4, 512], F32, tag="big", name=name)

        def transpose_512(xin, bt):
            """xin [128, 4, 512] f32r (4 x 128 rows of x) -> 4 SBUF tiles [128 d, 512 rows].

            Bank dc of the 4-bank slot bt collects the 4 transposes for d-chunk dc."""
            xts = []
            trv = bt[:].rearrange("p b (s c) -> p b s c", c=128)
            for dc in range(4):
                trb = trv[:, dc].bitcast(F32R)
                for s in range(4):
                    nc.tensor.transpose(trb[:, s, :], xin[:, s, dc * 128:(dc + 1) * 128],
                                        ident_r)
                xt = xtsp.tile([128, 512], F32R, tag="xt", name="xt")
                nc.vector.tensor_copy(xt, trb.rearrange("p a b -> p (a b)"))
                xts.append(xt)
            return xts

        # ---- Prelude: Q projection for this core's 1024 query rows ----
        with tc.tile_pool(name="wqp", bufs=1) as wqp:
            wq_sb = wqp.tile([128, 4, 512], F32R, tag="wq")
            nc.gpsimd.dma_start(out=wq_sb, in_=dram_chunks(wq_d))
            nc.gpsimd.dma_start(out=wk_sb, in_=dram_chunks(wk_d))
            nc.gpsimd.dma_start(out=wv_sb, in_=dram_chunks(wv_d))
            wo_st = wqp.tile([128, 4, 512], F32, tag="wo_st")
            nc.gpsimd.dma_start(out=wo_st, in_=dram_chunks(wo_d))
            nc.vector.tensor_copy(wo_sb, wo_st)
            for qch in range(NQ // 512):
                xq_sb = wqp.tile([128, 4, 512], F32R, tag="xq", name="xq_sb")
                nc.gpsimd.dma_start(out=xq_sb,
                                    in_=dram_chunks(xq_d)[:, qch * 4:(qch + 1) * 4, :])
                xts = transpose_512(xq_sb, big_tile("btq"))
                bq = big_tile("bq")
                for hc in range(4):
                    pq = bq[:, hc, :]
                    for dc in range(4):
                        nc.tensor.matmul(pq, wq_sb[:, dc, hc * 128:(hc + 1) * 128],
                                         xts[dc], start=(dc == 0), stop=(dc == 3))
                    nc.vector.tensor_copy(qT[hc][:, qch * 512:(qch + 1) * 512], pq)

        # ---- Fused K/V projection + attention over 512-token chunks ----
        vre = vsb[:]  # [128, NJC, H, 65]

        def emit_scores(qt, hp, ch, s):
            st = stp.tile([128, 1024], F32, tag="st", name="st")
            tok = ch * 512 + s * 128
            nc.tensor.matmul(st[:, 0:512], kT[hp][0:64, tok:tok + 128],
                             qT[hp][0:64, qt * 512:(qt + 1) * 512],
                             start=True, stop=True)
            nc.tensor.matmul(st[:, 512:1024], kT[hp][64:128, tok:tok + 128],
                             qT[hp][64:128, qt * 512:(qt + 1) * 512],
                             start=True, stop=True)
            ex = expp.tile([128, 1024], F16, tag="ex", name="ex")
            nc.scalar.activation(ex, st, EXP, scale=SCALE)
            return ex

        def emit_pv(pv, exs, hp, ch, phase, steps):
            """PV matmuls for phase (qc pair) at the given s steps; bank b = one
            accumulation group (qc-within-pair, head)."""
            for s in steps:
                j = ch * 4 + s
                for qp in range(2):
                    qc = phase * 2 + qp
                    for h2 in range(2):
                        nc.tensor.matmul(pv[:, 2 * qp + h2, 0:65],
                                         exs[s][:, h2 * 512 + qc * 128:
                                                h2 * 512 + (qc + 1) * 128],
                                         vre[:, j, 2 * hp + h2, :],
                                         start=(s == 0), stop=(s == 3),
                                         skip_group_check=True)

        def emit_drain(pv, qt, hp, ch, phase):
            dst = acc[:, qt, hp, phase, :].rearrange("p (a c) -> p a c", c=65)
            src = pv[:, :, 0:65]
            if ch == 0:
                nc.vector.tensor_copy(dst, src)
            else:
                nc.vector.tensor_add(dst, dst, src)

        def emit_epilogue(qt):
            """Normalize, transpose attn-out, project through Wo, store."""
            octiles = []
            bt = big_tile("bte")
            trv = bt[:].rearrange("p b (a c) -> p b a c", c=128)
            for hp in range(4):
                av = acc[:, qt, hp].rearrange("p t (a h c) -> p t a h c", h=2, c=65)
                rp = rppp.tile([128, 2, 2, 2, 1], F32, tag="rp", name="rp")
                nc.vector.reciprocal(rp, av[:, :, :, :, 64:65])
                ocn = ocnp.tile([128, 2, 2, 2, 64], F32R, tag="ocn", name="ocn")
                nc.vector.tensor_mul(ocn, av[:, :, :, :, 0:64],
                                     rp.broadcast_to((128, 2, 2, 2, 64)))
                ocf = ocn[:].rearrange("p t a h c -> p (t a) (h c)")  # [128, 4qc, 128]
                trq = trv[:, hp].bitcast(F32R)
                for qc in range(4):
                    nc.tensor.transpose(trq[:, qc, :], ocf[:, qc, :], ident_r)
                ocT = octp.tile([128, 512], F32R, tag="ocT", name="ocT")
                nc.vector.tensor_copy(ocT, trq.rearrange("p a b -> p (a b)"))
                octiles.append(ocT)
            ot = outpp.tile([128, 4, 512], F32, tag="ot", name="ot")
            bo_ = big_tile("bto")
            for it in range(4):
                po = bo_[:, it, :]
                for hp in range(4):
                    nc.tensor.matmul(po, octiles[hp][:, it * 128:(it + 1) * 128],
                                     wo_sb[:, hp, :], start=(hp == 0), stop=(hp == 3))
                nc.vector.tensor_add(ot[:, it, :], po, bo_bc)
            nc.gpsimd.dma_start(out=dram_chunks(out_d)[:, qt * 4:(qt + 1) * 4, :], in_=ot)

        for ch in range(NCH):
            # x chunk load + transpose + K/V projection (banks sub-aliased via WAR)
            xin = xinp.tile([128, 4, 512], F32R, tag="xin", name="xin")
            nc.gpsimd.dma_start(out=xin, in_=dram_chunks(x_d)[:, ch * 4:(ch + 1) * 4, :])
            bt = big_tile("btp")
            xts = transpose_512(xin, bt)
            bkv = big_tile("bkv")
            for hc in range(4):
                pk = bkv[:, hc, :]
                for dc in range(4):
                    nc.tensor.matmul(pk, wk_sb[:, dc, hc * 128:(hc + 1) * 128],
                                     xts[dc], start=(dc == 0), stop=(dc == 3))
                nc.vector.tensor_copy(kT[hc][:, ch * 512:(ch + 1) * 512], pk)
            bv = big_tile("bv")
            for s in range(4):
                pvj = bv[:, s, :]
                for dc in range(4):
                    nc.tensor.matmul(pvj, xts[dc][:, s * 128:(s + 1) * 128],
                                     wv_sb[:, dc, :], start=(dc == 0), stop=(dc == 3))
                nc.vector.tensor_copy(vre[:, ch * 4 + s, :, 0:64],
                                      pvj.rearrange("p (h c) -> p h c", c=64))

            # attention against this chunk's 512 tokens; software-pipelined so the
            # Activation engine never waits: scores run 1+ steps ahead of PV use.
            prev = None  # (pv, exs, qt, hp) with phase-1 (qc23) still to emit
            for qt in range(NQT):
                for hp in range(4):
                    pv = big_tile("pv")
                    exs = [emit_scores(qt, hp, ch, 0)]
                    if prev is not None:
                        emit_pv(prev[0], prev[1], prev[3], ch, 1, range(4))
                        emit_drain(prev[0], prev[2], prev[3], ch, 1)
                    exs.append(emit_scores(qt, hp, ch, 1))
                    emit_pv(pv, exs, hp, ch, 0, [0, 1])
                    exs.append(emit_scores(qt, hp, ch, 2))
                    emit_pv(pv, exs, hp, ch, 0, [2])
                    exs.append(emit_scores(qt, hp, ch, 3))
                    emit_pv(pv, exs, hp, ch, 0, [3])
                    emit_drain(pv, qt, hp, ch, 0)
                    prev = (pv, exs, qt, hp)
                # flush the pipeline at chunk end; overlap qt0's epilogue (last chunk)
                if qt == NQT - 1 or (ch == NCH - 1 and qt == 0):
                    emit_pv(prev[0], prev[1], prev[3], ch, 1, range(4))
                    emit_drain(prev[0], prev[2], prev[3], ch, 1)
                    prev = None
                    if ch == NCH - 1:
                        emit_epilogue(qt)

        if debug:
            nc.gpsimd.dma_start(out=dbg_kT[:], in_=kT[0][:])
            nc.gpsimd.dma_start(out=dbg_qT[:], in_=qT[0][:])
            nc.gpsimd.dma_start(out=dbg_v[:], in_=vsb[:].rearrange("p a b c -> p (a b c)"))
            nc.gpsimd.dma_start(out=dbg_acc[:],
                                in_=acc[:].rearrange("p a b c d -> p (a b c d)"))
    nc.finalize()
    return nc


_NC_CACHE = {}


def _get_nc(key="main"):
    if key not in _NC_CACHE:
        _NC_CACHE[key] = build_nc()
    return _NC_CACHE[key]


def _make_in_maps(inputs):
    x = np.ascontiguousarray(np.asarray(inputs["x"], dtype=np.float32))
    wq = np.ascontiguousarray(np.asarray(inputs["Wq"], dtype=np.float32))
    wk = np.ascontiguousarray(np.asarray(inputs["Wk"], dtype=np.float32))
    wv = np.ascontiguousarray(np.asarray(inputs["Wv"], dtype=np.float32))
    wo = np.ascontiguousarray(np.asarray(inputs["Wo"], dtype=np.float32))
    bo = np.ascontiguousarray(np.asarray(inputs["bo"], dtype=np.float32))
    in_maps = []
    for c in range(NCORES):
        b = c // CORES_PER_B
        r0 = (c % CORES_PER_B) * NQ
        in_maps.append({
            "x": np.ascontiguousarray(x[b]),
            "xq": np.ascontiguousarray(x[b, r0:r0 + NQ]),
            "wq": wq, "wk": wk, "wv": wv, "wo": wo, "bo": bo,
        })
    return in_maps


def _assemble(results):
    out = np.empty((B, N, D), dtype=np.float32)
    for c in range(NCORES):
        b = c // CORES_PER_B
        r0 = (c % CORES_PER_B) * NQ
        out[b, r0:r0 + NQ] = results[c]["out"]
    return out


def kernel(**inputs) -> np.ndarray:
    nc = _get_nc()
    res = run_bass_kernel_spmd(nc, _make_in_maps(inputs), core_ids=list(range(NCORES)))
    return _assemble(res.results)


def kernel_traced(**inputs):
    """Returns (output, exec_time_ns_or_None, result)."""
    nc = _get_nc()
    try:
        res = run_bass_kernel_spmd(nc, _make_in_maps(inputs), core_ids=list(range(NCORES)),
                                   trace=True)
    except (ModuleNotFoundError, ImportError):
        res = run_bass_kernel_spmd(nc, _make_in_maps(inputs), core_ids=list(range(NCORES)))
    return _assemble(res.results), res.exec_time_ns, res
